# revision 10
# baseline (speedup 1.0000x reference)
"""CRF loss (nn_ConditionalRandomField) Bass/Trainium2 kernel — v5.

Strategy
--------
loss = sum_b (numerator[b] - log_denominator[b])

The denominator is a length-512 sequential scan A_t = (W @ A_{t-1}) * E_t
(exp space, W = exp(transitions), E_t = exp(inputs_t) * 2^-C with a
constant per-step prescale C=9.42 — no data-dependent renorm; the exact
correction 512*C*ln2 is added back on the host).

The per-round latency cycle on TRN2 (PSUM-access vector op + sems +
matmul SBUF-read latency ~ 1.3us) is fixed hardware cost, so rounds are
cut by TIME SEGMENTATION: T=512 splits into K=8 segments of 64 steps.
Products of 64 positive transfer matrices are numerically rank-1
(Birkhoff contraction), so middle segments are exactly summarized by a
forward scan u_s = M_s @ 1 and a backward scan w_s = M_s^T @ 1:

  denom ~= ln[ (w2.y1) * prod_s (w_{s+1}.u_s) * (b8.u7) / prod_s (1.u_s) ]

All 7 forward chains share weights, so they pack into the free dim of
the SAME matmuls: state [128, 7, 64] (chain-major, j*32+b minor), FD=224
per matmul, ONE tensor_tensor [128, 7x64] per direction per round — the
[128,448] f32 PSUM tile sits in a single 2KB bank. 64 rounds total; the
fwd and bwd packs fill each other's latency gaps. At this point the
vector engine is ~95% busy: the kernel sits at the DVE-stream roofline
(1 elem/cycle for PSUM-source tensor_tensor).

Per core (batch-parallel, 32 sequences): per round 8 matmuls + 2 vector
multiplies. Weights fp8e4 (exp'd on host), state bf16, PSUM f32.
numerator is a tiny O(B*T) gather on the host.
"""

import numpy as np
import ml_dtypes

B, T, N = 256, 512, 256
START, STOP = 254, 255
NCORES = 8
BC = B // NCORES          # 32 sequences per core
K = 8                     # time segments
L = T // K                # 64 steps per segment = rounds
NCH = K - 1               # 7 chains per direction
FD = NCH * 2 * BC         # 448 packed state columns
SCALE_BITS = 9.42
LN2 = float(np.log(2.0))
# DMA chunk sizes in rounds (small first chunks -> scan starts fast)
CHUNKS = [1, 3, 4, 8, 8, 8, 8, 8, 8, 8]


def _build_program():
    import concourse.bass as bass  # noqa: F401
    import concourse.tile as tile
    from concourse import bacc, mybir

    f32 = mybir.dt.float32
    bf16 = mybir.dt.bfloat16
    fp8 = mybir.dt.float8e4

    nc = bacc.Bacc("TRN2", target_bir_lowering=False, debug=False,
                   enable_asserts=False)

    # E slabs: [p, round_slice, chain, j*32+b]; fwd slice i: seg1 (ci=0)
    # -> E'_{i+1}, middle seg s=ci+1 -> E'_{64ci+i}. bwd slice i (seg
    # s=ci+2) -> E'_{64ci+126-i}. Init values ride in `cinit`.
    ef = nc.dram_tensor("ef", [128, L, NCH, 2 * BC], bf16,
                        kind="ExternalInput").ap()
    eb = nc.dram_tensor("eb", [128, L, NCH, 2 * BC], bf16,
                        kind="ExternalInput").ap()
    cinit = nc.dram_tensor("cinit", [128, 2 * BC + FD], bf16,
                           kind="ExternalInput").ap()
    wall = nc.dram_tensor("wall", [128, 1024], fp8, kind="ExternalInput").ap()
    esst = nc.dram_tensor("esst", [128, 4], f32, kind="ExternalInput").ap()
    sums_out = nc.dram_tensor("sums", [1, (2 * NCH - 1) * 2 * BC], f32,
                              kind="ExternalOutput").ap()

    nchunks = len(CHUNKS)
    starts = np.cumsum([0] + CHUNKS).tolist()

    with tile.TileContext(nc) as tc:
        with (
            tc.tile_pool(name="consts", bufs=1) as consts,
            tc.tile_pool(name="ebig", bufs=1) as ebig,
            tc.tile_pool(name="afp", bufs=3) as afp,
            tc.tile_pool(name="bxp", bufs=3) as bxp,
            tc.tile_pool(name="fin", bufs=1) as fin,
            tc.tile_pool(name="psf", bufs=2, space="PSUM") as psfp,
            tc.tile_pool(name="psb", bufs=2, space="PSUM") as psbp,
            tc.tile_pool(name="pss1", bufs=1, space="PSUM") as pss1,
            tc.tile_pool(name="pss2", bufs=1, space="PSUM") as pss2,
        ):
            # ---- constants: merged, posted on the scalar queue ----
            cinit_t = consts.tile([128, 2 * BC + FD], bf16, tag="cinit")
            nc.scalar.dma_start(out=cinit_t, in_=cinit)
            esst_t = consts.tile([128, 4], f32, tag="esst")
            nc.scalar.dma_start(out=esst_t, in_=esst)
            wall_t = consts.tile([128, 1024], fp8, tag="wall")
            nc.scalar.dma_start(out=wall_t, in_=wall)

            ones128_bf = consts.tile([128, 1], bf16)
            nc.vector.memset(ones128_bf, 1.0)

            def wtf(i):
                return wall_t[:, i * 128:(i + 1) * 128]

            def wtb(i):
                return wall_t[:, 512 + i * 128:512 + (i + 1) * 128]

            # ---- E chunks (all on the sync queue, first ones tiny) ----
            efch, ebch = [None] * nchunks, [None] * nchunks
            for c in range(nchunks):
                t0, t1 = starts[c], starts[c + 1]
                n = t1 - t0
                e_ = ebig.tile([128, n, NCH, 2 * BC], bf16, tag=f"ef{c}")
                nc.sync.dma_start(out=e_, in_=ef[:, t0:t1, :, :])
                efch[c] = e_
                e_ = ebig.tile([128, n, NCH, 2 * BC], bf16, tag=f"eb{c}")
                nc.sync.dma_start(out=e_, in_=eb[:, t0:t1, :, :])
                ebch[c] = e_

            import bisect

            def _slice(chlist, i):
                c = bisect.bisect_right(starts, i) - 1
                return chlist[c][:, i - starts[c], :, :]

            def efs(i):
                return _slice(efch, i)

            def ebs(i):
                return _slice(ebch, i)

            # ---- init packed states ----
            af = afp.tile([128, NCH, 2 * BC], bf16, tag="af")
            for j in range(2):
                nc.vector.tensor_scalar_mul(
                    af[:, 0, j * BC:(j + 1) * BC],
                    cinit_t[:, j * BC:(j + 1) * BC], esst_t[:, j:j + 1])
            nc.vector.memset(af[:, 1:NCH, :], 1.0)
            bx = bxp.tile([128, NCH, 2 * BC], bf16, tag="bx")
            nc.vector.tensor_copy(
                bx[:, 0:NCH - 1, :],
                cinit_t[:, 2 * BC:2 * BC + (NCH - 1) * 2 * BC])
            for j in range(2):
                o = 2 * BC + (NCH - 1) * 2 * BC
                nc.vector.tensor_scalar_mul(
                    bx[:, NCH - 1, j * BC:(j + 1) * BC],
                    cinit_t[:, o + j * BC:o + (j + 1) * BC],
                    esst_t[:, 2 + j:3 + j])

            # ---- scan: 64 rounds ----
            af_prev = None
            for r in range(1, L + 1):
                psb = psbp.tile([128, NCH, 2 * BC], f32, tag="psb")
                for jo in range(2):
                    o = psb[:, :, jo * BC:(jo + 1) * BC]
                    nc.tensor.matmul(o, wtb(0 * 2 + jo), bx[:, :, 0:BC],
                                     start=True, stop=False)
                    nc.tensor.matmul(o, wtb(1 * 2 + jo), bx[:, :, BC:2 * BC],
                                     start=False, stop=True)
                if r <= L - 1:
                    bx_new = bxp.tile([128, NCH, 2 * BC], bf16, tag="bx")
                    nc.vector.tensor_mul(bx_new, psb, ebs(r - 1))
                    bx = bx_new
                    psf = psfp.tile([128, NCH, 2 * BC], f32, tag="psf")
                    for jo in range(2):
                        o = psf[:, :, jo * BC:(jo + 1) * BC]
                        nc.tensor.matmul(o, wtf(0 * 2 + jo), af[:, :, 0:BC],
                                         start=True, stop=False)
                        nc.tensor.matmul(o, wtf(1 * 2 + jo),
                                         af[:, :, BC:2 * BC],
                                         start=False, stop=True)
                    af_new = afp.tile([128, NCH, 2 * BC], bf16, tag="af")
                    nc.vector.tensor_mul(af_new, psf, efs(r - 1))
                    af = af_new
                else:
                    # last round: bwd MM only (psb = final betas); fwd
                    # advances only middle chains (seg1 stopped at y1),
                    # written IN PLACE so the join is one tensor_tensor.
                    psf = psfp.tile([128, NCH - 1, 2 * BC], f32, tag="psf")
                    for jo in range(2):
                        o = psf[:, :, jo * BC:(jo + 1) * BC]
                        nc.tensor.matmul(o, wtf(0 * 2 + jo),
                                         af[:, 1:NCH, 0:BC],
                                         start=True, stop=False)
                        nc.tensor.matmul(o, wtf(1 * 2 + jo),
                                         af[:, 1:NCH, BC:2 * BC],
                                         start=False, stop=True)
                    nc.vector.tensor_mul(af[:, 1:NCH, :], psf,
                                         efs(L - 1)[:, 1:NCH, :])

            # ---- join ----
            tj = fin.tile([128, NCH, 2 * BC], bf16, tag="tj")
            nc.vector.tensor_mul(tj, psb, af)
            s1 = pss1.tile([1, NCH, 2 * BC], f32, tag="s1")
            nc.tensor.matmul(s1, ones128_bf, tj, start=True, stop=True)
            s2 = pss2.tile([1, NCH - 1, 2 * BC], f32, tag="s2")
            nc.tensor.matmul(s2, ones128_bf, af[:, 1:NCH, :],
                             start=True, stop=True)
            sums_sb = fin.tile([1, (2 * NCH - 1) * 2 * BC], f32, tag="sums")
            nc.vector.tensor_copy(sums_sb[:, 0:FD], s1)
            nc.vector.tensor_copy(sums_sb[:, FD:], s2)
            nc.sync.dma_start(out=sums_out, in_=sums_sb)

    nc.compile()
    return nc


_PROG_CACHE = {}


def _get_program():
    if "p" not in _PROG_CACHE:
        _PROG_CACHE["p"] = _build_program()
    return _PROG_CACHE["p"]


def _host_numerator(inputs, transitions, tags, mask):
    fm = mask.astype(np.float32)
    score = transitions[tags[:, 0], START].astype(np.float32)
    trans_sc = transitions[tags[:, 1:], tags[:, :-1]] * fm[:, 1:]
    emit_sc = np.take_along_axis(
        inputs[:, :-1, :], tags[:, :-1, None], axis=2)[..., 0] * fm[:, :-1]
    score = score + trans_sc.sum(-1) + emit_sc.sum(-1)
    last_idx = (fm.sum(-1) - 1.0).astype(np.int32)
    last_tags = np.take_along_axis(tags, last_idx[:, None], axis=1)[:, 0]
    last_input = np.take_along_axis(
        inputs[:, -1, :], last_tags[:, None], axis=1)[:, 0]
    return score + transitions[STOP, last_tags] + last_input * fm[:, -1]


def _make_in_maps(inputs, transitions):
    bf = ml_dtypes.bfloat16
    fp8 = ml_dtypes.float8_e4m3

    ex = np.exp(inputs.astype(np.float32) - np.float32(SCALE_BITS * LN2))
    v = ex.transpose(2, 1, 0).reshape(2, 128, T, B)   # [j, p, t, b]

    tf = np.zeros((L, NCH), np.int64)
    tf[:L - 1, 0] = np.arange(1, L)
    for ci in range(1, NCH):
        tf[:, ci] = L * ci + np.arange(L)
    tb = np.zeros((L, NCH), np.int64)
    for ci in range(NCH):
        tb[:L - 1, ci] = L * ci + 2 * L - 2 - np.arange(L - 1)

    tc_ = np.maximum(transitions, -100.0).astype(np.float32)
    expt = np.exp(tc_)
    wfs = np.ascontiguousarray(
        expt.T.reshape(2, 128, 2, 128).transpose(0, 2, 1, 3)
    ).reshape(4, 128, 128)
    wbs = np.ascontiguousarray(
        expt.reshape(2, 128, 2, 128).transpose(0, 2, 1, 3)
    ).reshape(4, 128, 128)
    wall = np.concatenate([
        wfs.transpose(1, 0, 2).reshape(128, 512),
        wbs.transpose(1, 0, 2).reshape(128, 512)], axis=1).astype(fp8)
    esst = np.stack([
        np.exp(np.maximum(transitions[0:128, START], -100.0)),
        np.exp(np.maximum(transitions[128:256, START], -100.0)),
        np.exp(np.maximum(transitions[STOP, 0:128], -100.0)),
        np.exp(np.maximum(transitions[STOP, 128:256], -100.0)),
    ], axis=1).astype(np.float32)                     # [128, 4]

    in_maps = []
    for c in range(NCORES):
        vc = v[:, :, :, c * BC:(c + 1) * BC]          # [2, 128, T, BC]
        efc = np.ascontiguousarray(
            vc[:, :, tf, :].transpose(1, 2, 3, 0, 4)
        ).reshape(128, L, NCH, 2 * BC)
        efc[:, L - 1, 0, :] = 0.0
        ebc = np.ascontiguousarray(
            vc[:, :, tb, :].transpose(1, 2, 3, 0, 4)
        ).reshape(128, L, NCH, 2 * BC)
        ebc[:, L - 1, :, :] = 0.0
        efinit = np.ascontiguousarray(
            vc[:, :, 0, :].transpose(1, 0, 2)).reshape(128, 2 * BC)
        t_init = L * (np.arange(NCH) + 2) - 1         # E'_{64s-1}, s=2..8
        ebinit = np.ascontiguousarray(
            vc[:, :, t_init, :].transpose(1, 2, 0, 3)).reshape(128, FD)
        cin = np.concatenate([efinit, ebinit], axis=1)
        in_maps.append({
            "ef": efc.astype(bf), "eb": ebc.astype(bf),
            "cinit": cin.astype(bf), "wall": wall, "esst": esst,
        })
    return in_maps


def kernel(inputs, transitions, tags, mask, _trace=False):
    from concourse.bass_utils import run_bass_kernel_spmd

    inputs = np.asarray(inputs, dtype=np.float32)
    transitions = np.asarray(transitions, dtype=np.float32)
    tags = np.asarray(tags)
    mask = np.asarray(mask)

    nc = _get_program()
    in_maps = _make_in_maps(inputs, transitions)
    res = run_bass_kernel_spmd(nc, in_maps, list(range(NCORES)), trace=_trace)

    denoms = np.empty(B, np.float64)
    for c in range(NCORES):
        s = res.results[c]["sums"].reshape(-1).astype(np.float64)
        s1 = s[0:FD].reshape(NCH, 2, BC)
        s2 = s[FD:].reshape(NCH - 1, 2, BC)
        S1 = s1[:, 0, :] + s1[:, 1, :]                # (7, BC)
        S2 = s2[:, 0, :] + s2[:, 1, :]                # (6, BC)
        denoms[c * BC:(c + 1) * BC] = (
            np.log(S1).sum(0) - np.log(S2).sum(0)
            + T * SCALE_BITS * LN2)

    num = _host_numerator(inputs, transitions, tags, mask).astype(np.float64)
    out = np.float32(np.sum(num - denoms))
    if _trace:
        return out, res
    return out


# revision 11
# speedup vs baseline: 1.0104x; 1.0104x over previous
"""CRF loss (nn_ConditionalRandomField) Bass/Trainium2 kernel — v5.

Strategy
--------
loss = sum_b (numerator[b] - log_denominator[b])

The denominator is a length-512 sequential scan A_t = (W @ A_{t-1}) * E_t
(exp space, W = exp(transitions), E_t = exp(inputs_t) * 2^-C with a
constant per-step prescale C=9.42 — no data-dependent renorm; the exact
correction 512*C*ln2 is added back on the host).

The per-round latency cycle on TRN2 (PSUM-access vector op + sems +
matmul SBUF-read latency ~ 1.3us) is fixed hardware cost, so rounds are
cut by TIME SEGMENTATION: T=512 splits into K=8 segments of 64 steps.
Products of 64 positive transfer matrices are numerically rank-1
(Birkhoff contraction), so middle segments are exactly summarized by a
forward scan u_s = M_s @ 1 and a backward scan w_s = M_s^T @ 1:

  denom ~= ln[ (w2.y1) * prod_s (w_{s+1}.u_s) * (b8.u7) / prod_s (1.u_s) ]

All 7 forward chains share weights, so they pack into the free dim of
the SAME matmuls: state [128, 7, 64] (chain-major, j*32+b minor), FD=224
per matmul, ONE tensor_tensor [128, 7x64] per direction per round — the
[128,448] f32 PSUM tile sits in a single 2KB bank. 64 rounds total; the
fwd and bwd packs fill each other's latency gaps. At this point the
vector engine is ~95% busy: the kernel sits at the DVE-stream roofline
(1 elem/cycle for PSUM-source tensor_tensor).

Per core (batch-parallel, 32 sequences): per round 8 matmuls + 2 vector
multiplies. Weights fp8e4 (exp'd on host), state bf16, PSUM f32.
numerator is a tiny O(B*T) gather on the host.
"""

import numpy as np
import ml_dtypes

B, T, N = 256, 512, 256
START, STOP = 254, 255
NCORES = 8
BC = B // NCORES          # 32 sequences per core
K = 9                     # time segments: seg1 = 56 steps, segs 2..9 = 57
L = 57                    # rounds (= longest segment)
NCH = K - 1               # 8 chains per direction
FD = NCH * 2 * BC         # 512 packed state columns (one full PSUM bank)
SCALE_BITS = 9.42
LN2 = float(np.log(2.0))
# DMA chunk sizes in rounds (small first chunks -> scan starts fast)
CHUNKS = [1, 3, 4, 8, 8, 8, 8, 8, 9]


def _build_program():
    import concourse.bass as bass  # noqa: F401
    import concourse.tile as tile
    from concourse import bacc, mybir

    f32 = mybir.dt.float32
    bf16 = mybir.dt.bfloat16
    fp8 = mybir.dt.float8e4

    nc = bacc.Bacc("TRN2", target_bir_lowering=False, debug=False,
                   enable_asserts=False)

    # E slabs: [p, round_slice, chain, j*32+b]; fwd slice i: seg1 (ci=0)
    # -> E'_{i+1}, middle seg s=ci+1 -> E'_{64ci+i}. bwd slice i (seg
    # s=ci+2) -> E'_{64ci+126-i}. Init values ride in `cinit`.
    ef = nc.dram_tensor("ef", [128, L, NCH, 2 * BC], bf16,
                        kind="ExternalInput").ap()
    eb = nc.dram_tensor("eb", [128, L, NCH, 2 * BC], bf16,
                        kind="ExternalInput").ap()
    cinit = nc.dram_tensor("cinit", [128, 2 * BC + FD], bf16,
                           kind="ExternalInput").ap()
    wall = nc.dram_tensor("wall", [128, 1024], fp8, kind="ExternalInput").ap()
    esst = nc.dram_tensor("esst", [128, 4], f32, kind="ExternalInput").ap()
    sums_out = nc.dram_tensor("sums", [1, (2 * NCH - 1) * 2 * BC], f32,
                              kind="ExternalOutput").ap()

    nchunks = len(CHUNKS)
    starts = np.cumsum([0] + CHUNKS).tolist()

    with tile.TileContext(nc) as tc:
        with (
            tc.tile_pool(name="consts", bufs=1) as consts,
            tc.tile_pool(name="ebig", bufs=1) as ebig,
            tc.tile_pool(name="afp", bufs=3) as afp,
            tc.tile_pool(name="bxp", bufs=3) as bxp,
            tc.tile_pool(name="fin", bufs=1) as fin,
            tc.tile_pool(name="psf", bufs=2, space="PSUM") as psfp,
            tc.tile_pool(name="psb", bufs=2, space="PSUM") as psbp,
            tc.tile_pool(name="pss1", bufs=1, space="PSUM") as pss1,
            tc.tile_pool(name="pss2", bufs=1, space="PSUM") as pss2,
        ):
            # ---- constants: merged, posted on the scalar queue ----
            cinit_t = consts.tile([128, 2 * BC + FD], bf16, tag="cinit")
            nc.scalar.dma_start(out=cinit_t, in_=cinit)
            esst_t = consts.tile([128, 4], f32, tag="esst")
            nc.scalar.dma_start(out=esst_t, in_=esst)
            wall_t = consts.tile([128, 1024], fp8, tag="wall")
            nc.scalar.dma_start(out=wall_t, in_=wall)

            ones128_bf = consts.tile([128, 1], bf16)
            nc.vector.memset(ones128_bf, 1.0)

            def wtf(i):
                return wall_t[:, i * 128:(i + 1) * 128]

            def wtb(i):
                return wall_t[:, 512 + i * 128:512 + (i + 1) * 128]

            # ---- E chunks (all on the sync queue, first ones tiny) ----
            efch, ebch = [None] * nchunks, [None] * nchunks
            for c in range(nchunks):
                t0, t1 = starts[c], starts[c + 1]
                n = t1 - t0
                e_ = ebig.tile([128, n, NCH, 2 * BC], bf16, tag=f"ef{c}")
                nc.sync.dma_start(out=e_, in_=ef[:, t0:t1, :, :])
                efch[c] = e_
                e_ = ebig.tile([128, n, NCH, 2 * BC], bf16, tag=f"eb{c}")
                nc.sync.dma_start(out=e_, in_=eb[:, t0:t1, :, :])
                ebch[c] = e_

            import bisect

            def _slice(chlist, i):
                c = bisect.bisect_right(starts, i) - 1
                return chlist[c][:, i - starts[c], :, :]

            def efs(i):
                return _slice(efch, i)

            def ebs(i):
                return _slice(ebch, i)

            # ---- init packed states ----
            af = afp.tile([128, NCH, 2 * BC], bf16, tag="af")
            for j in range(2):
                nc.vector.tensor_scalar_mul(
                    af[:, 0, j * BC:(j + 1) * BC],
                    cinit_t[:, j * BC:(j + 1) * BC], esst_t[:, j:j + 1])
            nc.vector.memset(af[:, 1:NCH, :], 1.0)
            bx = bxp.tile([128, NCH, 2 * BC], bf16, tag="bx")
            nc.vector.tensor_copy(
                bx[:, 0:NCH - 1, :],
                cinit_t[:, 2 * BC:2 * BC + (NCH - 1) * 2 * BC])
            for j in range(2):
                o = 2 * BC + (NCH - 1) * 2 * BC
                nc.vector.tensor_scalar_mul(
                    bx[:, NCH - 1, j * BC:(j + 1) * BC],
                    cinit_t[:, o + j * BC:o + (j + 1) * BC],
                    esst_t[:, 2 + j:3 + j])

            # ---- scan: 64 rounds ----
            af_prev = None
            for r in range(1, L + 1):
                psb = psbp.tile([128, NCH, 2 * BC], f32, tag="psb")
                for jo in range(2):
                    o = psb[:, :, jo * BC:(jo + 1) * BC]
                    nc.tensor.matmul(o, wtb(0 * 2 + jo), bx[:, :, 0:BC],
                                     start=True, stop=False)
                    nc.tensor.matmul(o, wtb(1 * 2 + jo), bx[:, :, BC:2 * BC],
                                     start=False, stop=True)
                if r <= L - 1:
                    bx_new = bxp.tile([128, NCH, 2 * BC], bf16, tag="bx")
                    nc.vector.tensor_mul(bx_new, psb, ebs(r - 1))
                    bx = bx_new
                if r <= L - 2:
                    psf = psfp.tile([128, NCH, 2 * BC], f32, tag="psf")
                    for jo in range(2):
                        o = psf[:, :, jo * BC:(jo + 1) * BC]
                        nc.tensor.matmul(o, wtf(0 * 2 + jo), af[:, :, 0:BC],
                                         start=True, stop=False)
                        nc.tensor.matmul(o, wtf(1 * 2 + jo),
                                         af[:, :, BC:2 * BC],
                                         start=False, stop=True)
                    af_new = afp.tile([128, NCH, 2 * BC], bf16, tag="af")
                    nc.vector.tensor_mul(af_new, psf, efs(r - 1))
                    af = af_new
                else:
                    # rounds L-1, L: seg1 stopped at y1 = A_55; fwd
                    # advances only middle chains, written IN PLACE so
                    # the join is one tensor_tensor.
                    psf = psfp.tile([128, NCH - 1, 2 * BC], f32, tag="psf")
                    for jo in range(2):
                        o = psf[:, :, jo * BC:(jo + 1) * BC]
                        nc.tensor.matmul(o, wtf(0 * 2 + jo),
                                         af[:, 1:NCH, 0:BC],
                                         start=True, stop=False)
                        nc.tensor.matmul(o, wtf(1 * 2 + jo),
                                         af[:, 1:NCH, BC:2 * BC],
                                         start=False, stop=True)
                    nc.vector.tensor_mul(af[:, 1:NCH, :], psf,
                                         efs(r - 1)[:, 1:NCH, :])

            # ---- join ----
            tj = fin.tile([128, NCH, 2 * BC], bf16, tag="tj")
            nc.vector.tensor_mul(tj, psb, af)
            s1 = pss1.tile([1, NCH, 2 * BC], f32, tag="s1")
            nc.tensor.matmul(s1, ones128_bf, tj, start=True, stop=True)
            s2 = pss2.tile([1, NCH - 1, 2 * BC], f32, tag="s2")
            nc.tensor.matmul(s2, ones128_bf, af[:, 1:NCH, :],
                             start=True, stop=True)
            sums_sb = fin.tile([1, (2 * NCH - 1) * 2 * BC], f32, tag="sums")
            nc.vector.tensor_copy(sums_sb[:, 0:FD], s1)
            nc.vector.tensor_copy(sums_sb[:, FD:], s2)
            nc.sync.dma_start(out=sums_out, in_=sums_sb)

    nc.compile()
    return nc


_PROG_CACHE = {}


def _get_program():
    if "p" not in _PROG_CACHE:
        _PROG_CACHE["p"] = _build_program()
    return _PROG_CACHE["p"]


def _host_numerator(inputs, transitions, tags, mask):
    fm = mask.astype(np.float32)
    score = transitions[tags[:, 0], START].astype(np.float32)
    trans_sc = transitions[tags[:, 1:], tags[:, :-1]] * fm[:, 1:]
    emit_sc = np.take_along_axis(
        inputs[:, :-1, :], tags[:, :-1, None], axis=2)[..., 0] * fm[:, :-1]
    score = score + trans_sc.sum(-1) + emit_sc.sum(-1)
    last_idx = (fm.sum(-1) - 1.0).astype(np.int32)
    last_tags = np.take_along_axis(tags, last_idx[:, None], axis=1)[:, 0]
    last_input = np.take_along_axis(
        inputs[:, -1, :], last_tags[:, None], axis=1)[:, 0]
    return score + transitions[STOP, last_tags] + last_input * fm[:, -1]


def _make_in_maps(inputs, transitions):
    bf = ml_dtypes.bfloat16
    fp8 = ml_dtypes.float8_e4m3

    ex = np.exp(inputs.astype(np.float32) - np.float32(SCALE_BITS * LN2))
    v = ex.transpose(2, 1, 0).reshape(2, 128, T, B)   # [j, p, t, b]

    tf = np.zeros((L, NCH), np.int64)
    tf[:55, 0] = np.arange(1, 56)                 # seg1: E'_1..E'_55
    for ci in range(1, NCH):
        tf[:, ci] = 57 * ci - 1 + np.arange(L)    # seg s=ci+1: 57 steps
    tb = np.zeros((L, NCH), np.int64)
    for ci in range(NCH):
        tb[:L - 1, ci] = 111 + 57 * ci - np.arange(L - 1)

    tc_ = np.maximum(transitions, -100.0).astype(np.float32)
    expt = np.exp(tc_)
    wfs = np.ascontiguousarray(
        expt.T.reshape(2, 128, 2, 128).transpose(0, 2, 1, 3)
    ).reshape(4, 128, 128)
    wbs = np.ascontiguousarray(
        expt.reshape(2, 128, 2, 128).transpose(0, 2, 1, 3)
    ).reshape(4, 128, 128)
    wall = np.concatenate([
        wfs.transpose(1, 0, 2).reshape(128, 512),
        wbs.transpose(1, 0, 2).reshape(128, 512)], axis=1).astype(fp8)
    esst = np.stack([
        np.exp(np.maximum(transitions[0:128, START], -100.0)),
        np.exp(np.maximum(transitions[128:256, START], -100.0)),
        np.exp(np.maximum(transitions[STOP, 0:128], -100.0)),
        np.exp(np.maximum(transitions[STOP, 128:256], -100.0)),
    ], axis=1).astype(np.float32)                     # [128, 4]

    in_maps = []
    for c in range(NCORES):
        vc = v[:, :, :, c * BC:(c + 1) * BC]          # [2, 128, T, BC]
        efc = np.ascontiguousarray(
            vc[:, :, tf, :].transpose(1, 2, 3, 0, 4)
        ).reshape(128, L, NCH, 2 * BC)
        efc[:, 55:, 0, :] = 0.0
        ebc = np.ascontiguousarray(
            vc[:, :, tb, :].transpose(1, 2, 3, 0, 4)
        ).reshape(128, L, NCH, 2 * BC)
        ebc[:, L - 1, :, :] = 0.0
        efinit = np.ascontiguousarray(
            vc[:, :, 0, :].transpose(1, 0, 2)).reshape(128, 2 * BC)
        t_init = 112 + 57 * np.arange(NCH)            # E'_{112+57ci}, segs 2..9
        ebinit = np.ascontiguousarray(
            vc[:, :, t_init, :].transpose(1, 2, 0, 3)).reshape(128, FD)
        cin = np.concatenate([efinit, ebinit], axis=1)
        in_maps.append({
            "ef": efc.astype(bf), "eb": ebc.astype(bf),
            "cinit": cin.astype(bf), "wall": wall, "esst": esst,
        })
    return in_maps


def kernel(inputs, transitions, tags, mask, _trace=False):
    from concourse.bass_utils import run_bass_kernel_spmd

    inputs = np.asarray(inputs, dtype=np.float32)
    transitions = np.asarray(transitions, dtype=np.float32)
    tags = np.asarray(tags)
    mask = np.asarray(mask)

    nc = _get_program()
    in_maps = _make_in_maps(inputs, transitions)
    res = run_bass_kernel_spmd(nc, in_maps, list(range(NCORES)), trace=_trace)

    denoms = np.empty(B, np.float64)
    for c in range(NCORES):
        s = res.results[c]["sums"].reshape(-1).astype(np.float64)
        s1 = s[0:FD].reshape(NCH, 2, BC)
        s2 = s[FD:].reshape(NCH - 1, 2, BC)
        S1 = s1[:, 0, :] + s1[:, 1, :]                # (7, BC)
        S2 = s2[:, 0, :] + s2[:, 1, :]                # (6, BC)
        denoms[c * BC:(c + 1) * BC] = (
            np.log(S1).sum(0) - np.log(S2).sum(0)
            + T * SCALE_BITS * LN2)

    num = _host_numerator(inputs, transitions, tags, mask).astype(np.float64)
    out = np.float32(np.sum(num - denoms))
    if _trace:
        return out, res
    return out
